# revision 15
# baseline (speedup 1.0000x reference)
"""CommNet critic forward kernel for 8 trn2 NeuronCores.

Sharding: pure data parallel over the batch dim (B=2048 -> 256 per core).
Weights (<1MB) replicated. The agent-mean communication is within each
sample's 32-agent group, which never crosses a core boundary, so there are
no collectives.

On-chip layout is feature-major: activations live as [feature -> partition,
row -> free-dim] tiles, so
  * every matmul is out = W_T.T @ acts with the weight stationary,
  * biases are per-partition scalars fused into ACT/DVE instructions
    (or rank-1 ones-matmuls accumulated straight into PSUM),
  * the per-sample mean over 32 agents is a free-dim segmented reduction.
obs is transposed host-side (part of sharding prep) so the device never
transposes anything.

All matmul operands are bf16 (fp32 PSUM accumulate): on trn2 this is the
fast PE path (FWL weight loads + LDW/MM overlap); fp32/fp32r matmuls cost
3-4x more per instruction. Elementwise runs bf16 where operands live in
SBUF (2x DVE modes), with the GRU h-update chains offloaded to GpSimd.
"""

import sys

sys.path.insert(0, "/opt/trn_rl_repo")

import ml_dtypes
import numpy as np

import concourse.bacc as bacc
import concourse.mybir as mybir
import concourse.tile as tile
from concourse.bass_utils import run_bass_kernel_spmd

B, A, D, H = 2048, 32, 128, 256
NCORES = 8
B_LOC = B // NCORES          # 256 samples per core
N_LOC = B_LOC * A            # 8192 rows per core
R = 512                      # rows per tile (one PSUM bank of fp32)

F32 = mybir.dt.float32
BF16 = mybir.dt.bfloat16
NP_BF16 = ml_dtypes.bfloat16

AF = mybir.ActivationFunctionType
OP = mybir.AluOpType

S0 = slice(0, 512)
S1 = slice(512, 1024)


def build_nc(n_rows=N_LOC):
    assert n_rows % R == 0
    nt = n_rows // R
    nc = bacc.Bacc("TRN2", target_bir_lowering=False, debug=False)

    xT = nc.declare_dram_parameter("xT", [D, n_rows], BF16, isOutput=False)
    encT = nc.declare_dram_parameter("encT", [128, 256], BF16, isOutput=False)
    fobsT = nc.declare_dram_parameter("fobsT", [128, 512], BF16, isOutput=False)
    whhT = nc.declare_dram_parameter("whhT", [128, 1536], BF16, isOutput=False)
    wihT = nc.declare_dram_parameter("wihT", [128, 1536], BF16, isOutput=False)
    decT = nc.declare_dram_parameter("decT", [128, 2], BF16, isOutput=False)
    encb = nc.declare_dram_parameter("encb", [128, 2], F32, isOutput=False)
    fobsb = nc.declare_dram_parameter("fobsb", [128, 2], F32, isOutput=False)
    brz = nc.declare_dram_parameter("brz", [128, 4], F32, isOutput=False)
    bhn = nc.declare_dram_parameter("bhn", [128, 2], F32, isOutput=False)
    binb = nc.declare_dram_parameter("binb", [128, 2], F32, isOutput=False)
    decb = nc.declare_dram_parameter("decb", [1, 1], F32, isOutput=False)
    out = nc.declare_dram_parameter("out", [1, n_rows], F32, isOutput=True)

    def mm(o, lhsT, rhs, start, stop):
        nc.tensor.matmul(o, lhsT, rhs, start=start, stop=stop)

    with tile.TileContext(nc, pool_alloc_mode="queue") as tc:
        with (
            tc.tile_pool(name="wpool", bufs=1) as wp,
            tc.tile_pool(name="io", bufs=4) as io,
            tc.tile_pool(name="acts", bufs=4) as ap,
            tc.tile_pool(name="psum", bufs=8, space="PSUM") as pp,
        ):
            encT_s = wp.tile([128, 256], BF16, name="encT_s", tag="encT_s")
            fobsT_s = wp.tile([128, 512], BF16, name="fobsT_s", tag="fobsT_s")
            whhT_s = wp.tile([128, 1536], BF16, name="whhT_s", tag="whhT_s")
            wihT_s = wp.tile([128, 1536], BF16, name="wihT_s", tag="wihT_s")
            decT_s = wp.tile([128, 2], BF16, name="decT_s", tag="decT_s")
            encb_s = wp.tile([128, 2], F32, name="encb_s", tag="encb_s")
            fobsb_s = wp.tile([128, 2], F32, name="fobsb_s", tag="fobsb_s")
            brz_s = wp.tile([128, 4], F32, name="brz_s", tag="brz_s")
            bhn_s = wp.tile([128, 2], F32, name="bhn_s", tag="bhn_s")
            binb_s = wp.tile([128, 2], F32, name="binb_s", tag="binb_s")
            decb_s = wp.tile([1, 1], F32, name="decb_s", tag="decb_s")
            for t, d in [
                (encT_s, encT), (fobsT_s, fobsT), (whhT_s, whhT),
                (wihT_s, wihT), (decT_s, decT), (encb_s, encb),
                (fobsb_s, fobsb), (brz_s, brz), (bhn_s, bhn),
                (binb_s, binb), (decb_s, decb),
            ]:
                nc.sync.dma_start(t[:], d.ap())

            xT_ap = xT.ap()
            out_ap = out.ap()

            # Software pipeline: PE stream is A(t+2) | B(t) | C(t-1) so every
            # matmul's inputs were produced 1-2 phases earlier and the PE
            # never waits on the elementwise chain (keeps HAM warm, K=8/8).
            #   A: enc + fobs + GRU1 + comm  -> h1, cp
            #   B: GRU2                       -> h2
            #   C: dec + output DMA
            st = {}

            def emitA(t):
                r0 = t * R
                xt = io.tile([128, R], BF16, name="xt", tag="xt")
                nc.sync.dma_start(xt[:], xT_ap[:, r0 : r0 + R])

                # encoder: e = relu(enc_W @ x + enc_b)
                pe = [pp.tile([128, 512], F32, name=f"pe{m}", tag="ps") for m in (0, 1)]
                for m in (0, 1):
                    mm(pe[m][:], encT_s[:, 128 * m : 128 * m + 128], xt[:], True, True)
                e = ap.tile([128, 1024], BF16, name="e", tag="e")
                for m, sl in ((0, S0), (1, S1)):
                    nc.scalar.activation(
                        e[:, sl], pe[m][:], AF.Relu, bias=encb_s[:, m : m + 1]
                    )

                # fobs: h0 = fobs_W @ e + fobs_b
                ph = [pp.tile([128, 512], F32, name=f"ph{m}", tag="ps") for m in (0, 1)]
                for m in (0, 1):
                    for k in (0, 1):
                        mm(
                            ph[m][:],
                            fobsT_s[:, 256 * k + 128 * m : 256 * k + 128 * m + 128],
                            e[:, 512 * k : 512 * k + 512],
                            k == 0,
                            k == 1,
                        )
                h0 = ap.tile([128, 1024], BF16, name="h0", tag="h0")
                for m, sl in ((0, S0), (1, S1)):
                    nc.vector.tensor_scalar_add(h0[:, sl], ph[m][:], fobsb_s[:, m : m + 1])

                # GRU1 (x = 0 so gi = b_ih); gates packed [r | z | n]
                prz = [pp.tile([128, 512], F32, name=f"prz{g}", tag="ps") for g in range(4)]
                for g in range(4):
                    for k in (0, 1):
                        mm(
                            prz[g][:],
                            whhT_s[:, 768 * k + 128 * g : 768 * k + 128 * g + 128],
                            h0[:, 512 * k : 512 * k + 512],
                            k == 0,
                            k == 1,
                        )
                pn = [pp.tile([128, 512], F32, name=f"pn{m}", tag="ps") for m in (0, 1)]
                for m in (0, 1):
                    for k in (0, 1):
                        mm(
                            pn[m][:],
                            whhT_s[:, 768 * k + 512 + 128 * m : 768 * k + 640 + 128 * m],
                            h0[:, 512 * k : 512 * k + 512],
                            k == 0,
                            k == 1,
                        )
                rz1 = ap.tile([128, 2048], BF16, name="rz1", tag="rz1")
                for g in range(4):
                    nc.scalar.activation(
                        rz1[:, 512 * g : 512 * g + 512],
                        prz[g][:],
                        AF.Sigmoid,
                        bias=brz_s[:, g : g + 1],
                    )
                # n1 = tanh(b_ih_n + r1 * (gh_n + b_hh_n))
                tmp1 = ap.tile([128, 1024], BF16, name="tmp1", tag="tmp1")
                for m, sl in ((0, S0), (1, S1)):
                    nc.vector.scalar_tensor_tensor(
                        tmp1[:, sl], pn[m][:], bhn_s[:, m : m + 1],
                        rz1[:, sl], OP.add, OP.mult,
                    )
                n1 = ap.tile([128, 1024], BF16, name="n1", tag="n1")
                for m, sl in ((0, S0), (1, S1)):
                    nc.scalar.activation(
                        n1[:, sl], tmp1[:, sl], AF.Tanh, bias=binb_s[:, m : m + 1]
                    )
                # h1 = n1 + z1*(h0 - n1)
                d1 = ap.tile([128, 1024], BF16, name="d1", tag="d1")
                nc.vector.tensor_sub(d1[:], h0[:], n1[:])
                m1 = ap.tile([128, 1024], BF16, name="m1", tag="m1")
                nc.gpsimd.tensor_mul(m1[:], rz1[:, 1024:2048], d1[:])
                h1 = ap.tile([128, 1024], BF16, name="h1", tag="h1", bufs=6)
                nc.vector.tensor_add(h1[:], n1[:], m1[:])

                # comm: c' = (sum_group h1) - h1  (1/A folded into W_ih)
                S = ap.tile([128, 32], F32, name="S", tag="S")
                nc.vector.tensor_reduce(
                    S[:].rearrange("p (h g) -> p h g", h=2),
                    h1[:].rearrange("p (h g a) -> p h g a", h=2, a=32),
                    mybir.AxisListType.X,
                    OP.add,
                )
                cp = ap.tile([128, 1024], BF16, name="cp", tag="cp", bufs=6)
                Sb = (
                    S[:]
                    .rearrange("p (h g) -> p h g", h=2)
                    .unsqueeze(-1)
                    .broadcast_to([128, 2, 16, 32])
                )
                nc.gpsimd.tensor_tensor(
                    cp[:].rearrange("p (h g a) -> p h g a", h=2, a=32),
                    Sb,
                    h1[:].rearrange("p (h g a) -> p h g a", h=2, a=32),
                    OP.subtract,
                )
                st[t] = {"h1": h1, "cp": cp}

            def emitB(t):
                h1, cp = st[t]["h1"], st[t]["cp"]
                # GRU2: gi = (W_ih/A) @ c' + b_ih ; gh = W_hh @ h1 + b_hh
                prz2 = [pp.tile([128, 512], F32, name=f"prz2{g}", tag="ps") for g in range(4)]
                for g in range(4):
                    w0 = 128 * g
                    mm(prz2[g][:], wihT_s[:, w0 : w0 + 128], cp[:, S0], True, False)
                    mm(prz2[g][:], wihT_s[:, 768 + w0 : 768 + w0 + 128], cp[:, S1], False, False)
                    mm(prz2[g][:], whhT_s[:, w0 : w0 + 128], h1[:, S0], False, False)
                    mm(prz2[g][:], whhT_s[:, 768 + w0 : 768 + w0 + 128], h1[:, S1], False, True)
                phn = [pp.tile([128, 512], F32, name=f"phn{m}", tag="ps") for m in (0, 1)]
                pin = [pp.tile([128, 512], F32, name=f"pin{m}", tag="ps") for m in (0, 1)]
                for m in (0, 1):
                    for k in (0, 1):
                        mm(
                            phn[m][:],
                            whhT_s[:, 768 * k + 512 + 128 * m : 768 * k + 640 + 128 * m],
                            h1[:, 512 * k : 512 * k + 512],
                            k == 0,
                            k == 1,
                        )
                for m in (0, 1):
                    for k in (0, 1):
                        mm(
                            pin[m][:],
                            wihT_s[:, 768 * k + 512 + 128 * m : 768 * k + 640 + 128 * m],
                            cp[:, 512 * k : 512 * k + 512],
                            k == 0,
                            k == 1,
                        )
                rz2 = ap.tile([128, 2048], BF16, name="rz2", tag="rz2")
                for g in range(4):
                    nc.scalar.activation(
                        rz2[:, 512 * g : 512 * g + 512],
                        prz2[g][:],
                        AF.Sigmoid,
                        bias=brz_s[:, g : g + 1],
                    )
                # n2 = tanh(b_ih_n + i_n + r2 * (gh_n + b_hh_n))
                tmp2 = ap.tile([128, 1024], BF16, name="tmp2", tag="tmp2")
                for m, sl in ((0, S0), (1, S1)):
                    nc.vector.scalar_tensor_tensor(
                        tmp2[:, sl], phn[m][:], bhn_s[:, m : m + 1],
                        rz2[:, sl], OP.add, OP.mult,
                    )
                s2 = ap.tile([128, 1024], BF16, name="s2", tag="s2")
                for m, sl in ((0, S0), (1, S1)):
                    nc.vector.tensor_add(s2[:, sl], tmp2[:, sl], pin[m][:])
                n2 = ap.tile([128, 1024], BF16, name="n2", tag="n2")
                for m, sl in ((0, S0), (1, S1)):
                    nc.scalar.activation(
                        n2[:, sl], s2[:, sl], AF.Tanh, bias=binb_s[:, m : m + 1]
                    )
                # h2 = n2 + z2*(h1 - n2)
                d2 = ap.tile([128, 1024], BF16, name="d2", tag="d2")
                nc.vector.tensor_sub(d2[:], h1[:], n2[:])
                m2 = ap.tile([128, 1024], BF16, name="m2", tag="m2")
                nc.gpsimd.tensor_mul(m2[:], rz2[:, 1024:2048], d2[:])
                h2 = ap.tile([128, 1024], BF16, name="h2", tag="h2")
                nc.vector.tensor_add(h2[:], n2[:], m2[:])
                st[t]["h2"] = h2

            def emitC(t):
                h2 = st.pop(t)["h2"]
                r0 = t * R
                pd = pp.tile([1, 512], F32, name="pd", tag="ps")
                mm(pd[:], decT_s[:, 0:1], h2[:, S0], True, False)
                mm(pd[:], decT_s[:, 1:2], h2[:, S1], False, True)
                ot = io.tile([1, 512], F32, name="ot", tag="ot")
                nc.vector.tensor_scalar_add(ot[:], pd[:], decb_s[0:1, 0:1])
                nc.sync.dma_start(out_ap[0:1, r0 : r0 + R], ot[:])

            emitA(0)
            emitA(1)
            for t in range(nt):
                emitB(t)
                if t + 2 < nt:
                    emitA(t + 2)
                if t >= 1:
                    emitC(t - 1)
            emitC(nt - 1)

    nc.compile()
    return nc


def prep_shared(enc_W, enc_b, fobs_W, fobs_b, W_ih, b_ih, W_hh, b_hh, dec_W, dec_b):
    f = np.float32
    whhT = W_hh.T.astype(f)                      # [256, 768]
    wihT = (W_ih / A).T.astype(f)                # [256, 768], 1/A folded in
    bsum = (b_ih + b_hh).astype(f)
    bf = NP_BF16
    return {
        "encT": np.ascontiguousarray(enc_W.T).astype(bf),                    # [128,256]
        "fobsT": np.ascontiguousarray(
            np.concatenate([fobs_W.T[0:128], fobs_W.T[128:256]], axis=1)
        ).astype(bf),                                                        # [128,512]
        "whhT": np.ascontiguousarray(
            np.concatenate([whhT[0:128], whhT[128:256]], axis=1)
        ).astype(bf),                                                        # [128,1536]
        "wihT": np.ascontiguousarray(
            np.concatenate([wihT[0:128], wihT[128:256]], axis=1)
        ).astype(bf),                                                        # [128,1536]
        "decT": np.ascontiguousarray(
            np.concatenate([dec_W.T[0:128], dec_W.T[128:256]], axis=1)
        ).astype(bf),                                                        # [128,2]
        "encb": np.ascontiguousarray(enc_b.reshape(2, 128).T.astype(f)),
        "fobsb": np.ascontiguousarray(fobs_b.reshape(2, 128).T.astype(f)),
        "brz": np.ascontiguousarray(bsum[0:512].reshape(4, 128).T),
        "bhn": np.ascontiguousarray(b_hh[512:768].reshape(2, 128).T.astype(f)),
        "binb": np.ascontiguousarray(b_ih[512:768].reshape(2, 128).T.astype(f)),
        "decb": dec_b.reshape(1, 1).astype(f),
    }


_NC_CACHE = {}


def _get_nc(n_rows):
    if n_rows not in _NC_CACHE:
        _NC_CACHE[n_rows] = build_nc(n_rows)
    return _NC_CACHE[n_rows]


def run(inputs, trace=False):
    """Shard, run on 8 cores, gather. Returns (out [B,A,1] f32, results)."""
    obs = np.asarray(inputs["obs"], dtype=np.float32)
    shared = prep_shared(
        np.asarray(inputs["enc_W"]), np.asarray(inputs["enc_b"]),
        np.asarray(inputs["fobs_W"]), np.asarray(inputs["fobs_b"]),
        np.asarray(inputs["W_ih"]), np.asarray(inputs["b_ih"]),
        np.asarray(inputs["W_hh"]), np.asarray(inputs["b_hh"]),
        np.asarray(inputs["dec_W"]), np.asarray(inputs["dec_b"]),
    )
    in_maps = []
    for c in range(NCORES):
        xT = np.ascontiguousarray(
            obs[c * B_LOC : (c + 1) * B_LOC].reshape(N_LOC, D).T
        ).astype(NP_BF16)
        in_maps.append({"xT": xT, **shared})

    nc = _get_nc(N_LOC)
    res = run_bass_kernel_spmd(nc, in_maps, core_ids=list(range(NCORES)), trace=trace)
    outs = [res.results[c]["out"].reshape(N_LOC) for c in range(NCORES)]
    full = np.concatenate(outs).reshape(B, A, 1).astype(np.float32)
    return full, res


def kernel(**inputs):
    out, _ = run(inputs, trace=False)
    return out


# revision 16
# speedup vs baseline: 1.1968x; 1.1968x over previous
"""CommNet critic forward kernel for 8 trn2 NeuronCores.

Sharding: pure data parallel over the batch dim (B=2048 -> 256 per core).
Weights (<1MB) replicated. The agent-mean communication is within each
sample's 32-agent group, which never crosses a core boundary, so there are
no collectives.

On-chip layout is feature-major: activations live as [feature -> partition,
row -> free-dim] tiles, so
  * every matmul is out = W_T.T @ acts with the weight stationary,
  * biases are per-partition scalars fused into ACT/DVE instructions
    (or rank-1 ones-matmuls accumulated straight into PSUM),
  * the per-sample mean over 32 agents is a free-dim segmented reduction.
obs is transposed host-side (part of sharding prep) so the device never
transposes anything.

All matmul operands are bf16 (fp32 PSUM accumulate): on trn2 this is the
fast PE path (FWL weight loads + LDW/MM overlap); fp32/fp32r matmuls cost
3-4x more per instruction. Elementwise runs bf16 where operands live in
SBUF (2x DVE modes), with the GRU h-update chains offloaded to GpSimd.
"""

import sys

sys.path.insert(0, "/opt/trn_rl_repo")

import ml_dtypes
import numpy as np

import concourse.bacc as bacc
import concourse.mybir as mybir
import concourse.tile as tile
from concourse.bass_utils import run_bass_kernel_spmd

B, A, D, H = 2048, 32, 128, 256
NCORES = 8
B_LOC = B // NCORES          # 256 samples per core
N_LOC = B_LOC * A            # 8192 rows per core
R = 512                      # rows per tile (one PSUM bank of fp32)

F32 = mybir.dt.float32
BF16 = mybir.dt.bfloat16
NP_BF16 = ml_dtypes.bfloat16

AF = mybir.ActivationFunctionType
OP = mybir.AluOpType

S0 = slice(0, 512)
S1 = slice(512, 1024)


def build_nc(n_rows=N_LOC):
    assert n_rows % R == 0
    nt = n_rows // R
    nc = bacc.Bacc("TRN2", target_bir_lowering=False, debug=False)

    xT = nc.declare_dram_parameter("xT", [D, n_rows], BF16, isOutput=False)
    encT = nc.declare_dram_parameter("encT", [128, 256], BF16, isOutput=False)
    fobsT = nc.declare_dram_parameter("fobsT", [128, 512], BF16, isOutput=False)
    whhT = nc.declare_dram_parameter("whhT", [128, 1536], BF16, isOutput=False)
    wihT = nc.declare_dram_parameter("wihT", [128, 1536], BF16, isOutput=False)
    decT = nc.declare_dram_parameter("decT", [128, 2], BF16, isOutput=False)
    encb = nc.declare_dram_parameter("encb", [128, 2], F32, isOutput=False)
    fobsb = nc.declare_dram_parameter("fobsb", [128, 2], F32, isOutput=False)
    brz = nc.declare_dram_parameter("brz", [128, 4], F32, isOutput=False)
    bhn = nc.declare_dram_parameter("bhn", [128, 2], F32, isOutput=False)
    binb = nc.declare_dram_parameter("binb", [128, 2], F32, isOutput=False)
    decb = nc.declare_dram_parameter("decb", [1, 1], F32, isOutput=False)
    out = nc.declare_dram_parameter("out", [1, n_rows], F32, isOutput=True)

    def mm(o, lhsT, rhs, start, stop):
        nc.tensor.matmul(o, lhsT, rhs, start=start, stop=stop)

    with tile.TileContext(nc, pool_alloc_mode="queue") as tc:
        with (
            tc.tile_pool(name="wpool", bufs=1) as wp,
            tc.tile_pool(name="io", bufs=4) as io,
            tc.tile_pool(name="acts", bufs=4) as ap,
            tc.tile_pool(name="psum", bufs=7, space="PSUM") as pp,
            tc.tile_pool(name="psumd", bufs=1, space="PSUM") as ppd,
        ):
            encT_s = wp.tile([128, 256], BF16, name="encT_s", tag="encT_s")
            fobsT_s = wp.tile([128, 512], BF16, name="fobsT_s", tag="fobsT_s")
            whhT_s = wp.tile([128, 1536], BF16, name="whhT_s", tag="whhT_s")
            wihT_s = wp.tile([128, 1536], BF16, name="wihT_s", tag="wihT_s")
            decT_s = wp.tile([128, 2], BF16, name="decT_s", tag="decT_s")
            encb_s = wp.tile([128, 2], F32, name="encb_s", tag="encb_s")
            fobsb_s = wp.tile([128, 2], F32, name="fobsb_s", tag="fobsb_s")
            brz_s = wp.tile([128, 4], F32, name="brz_s", tag="brz_s")
            bhn_s = wp.tile([128, 2], F32, name="bhn_s", tag="bhn_s")
            binb_s = wp.tile([128, 2], F32, name="binb_s", tag="binb_s")
            decb_s = wp.tile([1, 1], F32, name="decb_s", tag="decb_s")
            for t, d in [
                (encT_s, encT), (fobsT_s, fobsT), (whhT_s, whhT),
                (wihT_s, wihT), (decT_s, decT), (encb_s, encb),
                (fobsb_s, fobsb), (brz_s, brz), (bhn_s, bhn),
                (binb_s, binb), (decb_s, decb),
            ]:
                nc.sync.dma_start(t[:], d.ap())

            xT_ap = xT.ap()
            out_ap = out.ap()

            # Software pipeline: PE stream is A(t+2) | B(t) | C(t-1) so every
            # matmul's inputs were produced 1-2 phases earlier and the PE
            # never waits on the elementwise chain (keeps HAM warm, K=8/8).
            #   A: enc + fobs + GRU1 + comm  -> h1, cp
            #   B: GRU2                       -> h2
            #   C: dec + output DMA
            st = {}

            def emitA(t):
                r0 = t * R
                xt = io.tile([128, R], BF16, name="xt", tag="xt")
                nc.sync.dma_start(xt[:], xT_ap[:, r0 : r0 + R])

                # encoder: e = relu(enc_W @ x + enc_b)
                pe = [pp.tile([128, 512], F32, name=f"pe{m}", tag="ps") for m in (0, 1)]
                for m in (0, 1):
                    mm(pe[m][:], encT_s[:, 128 * m : 128 * m + 128], xt[:], True, True)
                e = ap.tile([128, 1024], BF16, name="e", tag="e")
                for m, sl in ((0, S0), (1, S1)):
                    nc.scalar.activation(
                        e[:, sl], pe[m][:], AF.Relu, bias=encb_s[:, m : m + 1]
                    )

                # fobs: h0 = fobs_W @ e + fobs_b
                ph = [pp.tile([128, 512], F32, name=f"ph{m}", tag="ps") for m in (0, 1)]
                for m in (0, 1):
                    for k in (0, 1):
                        mm(
                            ph[m][:],
                            fobsT_s[:, 256 * k + 128 * m : 256 * k + 128 * m + 128],
                            e[:, 512 * k : 512 * k + 512],
                            k == 0,
                            k == 1,
                        )
                h0 = ap.tile([128, 1024], BF16, name="h0", tag="h0")
                for m, sl in ((0, S0), (1, S1)):
                    nc.vector.tensor_scalar_add(h0[:, sl], ph[m][:], fobsb_s[:, m : m + 1])

                # GRU1 (x = 0 so gi = b_ih); gates packed [r | z | n]
                prz = [pp.tile([128, 512], F32, name=f"prz{g}", tag="ps") for g in range(4)]
                for g in range(4):
                    for k in (0, 1):
                        mm(
                            prz[g][:],
                            whhT_s[:, 768 * k + 128 * g : 768 * k + 128 * g + 128],
                            h0[:, 512 * k : 512 * k + 512],
                            k == 0,
                            k == 1,
                        )
                pn = [pp.tile([128, 512], F32, name=f"pn{m}", tag="ps") for m in (0, 1)]
                for m in (0, 1):
                    for k in (0, 1):
                        mm(
                            pn[m][:],
                            whhT_s[:, 768 * k + 512 + 128 * m : 768 * k + 640 + 128 * m],
                            h0[:, 512 * k : 512 * k + 512],
                            k == 0,
                            k == 1,
                        )
                rz1 = ap.tile([128, 2048], BF16, name="rz1", tag="rz1")
                for g in range(4):
                    nc.scalar.activation(
                        rz1[:, 512 * g : 512 * g + 512],
                        prz[g][:],
                        AF.Sigmoid,
                        bias=brz_s[:, g : g + 1],
                    )
                # n1 = tanh(b_ih_n + r1 * (gh_n + b_hh_n))
                tmp1 = ap.tile([128, 1024], BF16, name="tmp1", tag="tmp1")
                for m, sl in ((0, S0), (1, S1)):
                    nc.vector.scalar_tensor_tensor(
                        tmp1[:, sl], pn[m][:], bhn_s[:, m : m + 1],
                        rz1[:, sl], OP.add, OP.mult,
                    )
                n1 = ap.tile([128, 1024], BF16, name="n1", tag="n1")
                for m, sl in ((0, S0), (1, S1)):
                    nc.scalar.activation(
                        n1[:, sl], tmp1[:, sl], AF.Tanh, bias=binb_s[:, m : m + 1]
                    )
                # h1 = n1 + z1*(h0 - n1)
                d1 = ap.tile([128, 1024], BF16, name="d1", tag="d1")
                nc.vector.tensor_sub(d1[:], h0[:], n1[:])
                m1 = ap.tile([128, 1024], BF16, name="m1", tag="m1")
                nc.gpsimd.tensor_mul(m1[:], rz1[:, 1024:2048], d1[:])
                h1 = ap.tile([128, 1024], BF16, name="h1", tag="h1", bufs=6)
                nc.vector.tensor_add(h1[:], n1[:], m1[:])

                # comm: c' = (sum_group h1) - h1  (1/A folded into W_ih)
                S = ap.tile([128, 32], F32, name="S", tag="S")
                nc.vector.tensor_reduce(
                    S[:].rearrange("p (h g) -> p h g", h=2),
                    h1[:].rearrange("p (h g a) -> p h g a", h=2, a=32),
                    mybir.AxisListType.X,
                    OP.add,
                )
                cp = ap.tile([128, 1024], BF16, name="cp", tag="cp", bufs=6)
                Sb = (
                    S[:]
                    .rearrange("p (h g) -> p h g", h=2)
                    .unsqueeze(-1)
                    .broadcast_to([128, 2, 16, 32])
                )
                nc.gpsimd.tensor_tensor(
                    cp[:].rearrange("p (h g a) -> p h g a", h=2, a=32),
                    Sb,
                    h1[:].rearrange("p (h g a) -> p h g a", h=2, a=32),
                    OP.subtract,
                )
                st[t] = {"h1": h1, "cp": cp}

            def emitB(t):
                h1, cp = st[t]["h1"], st[t]["cp"]
                # GRU2: gi = (W_ih/A) @ c' + b_ih ; gh = W_hh @ h1 + b_hh
                prz2 = [pp.tile([128, 512], F32, name=f"prz2{g}", tag="ps") for g in range(4)]
                for g in range(4):
                    w0 = 128 * g
                    mm(prz2[g][:], wihT_s[:, w0 : w0 + 128], cp[:, S0], True, False)
                    mm(prz2[g][:], wihT_s[:, 768 + w0 : 768 + w0 + 128], cp[:, S1], False, False)
                    mm(prz2[g][:], whhT_s[:, w0 : w0 + 128], h1[:, S0], False, False)
                    mm(prz2[g][:], whhT_s[:, 768 + w0 : 768 + w0 + 128], h1[:, S1], False, True)
                phn = [pp.tile([128, 512], F32, name=f"phn{m}", tag="ps") for m in (0, 1)]
                pin = [pp.tile([128, 512], F32, name=f"pin{m}", tag="ps") for m in (0, 1)]
                for m in (0, 1):
                    for k in (0, 1):
                        mm(
                            phn[m][:],
                            whhT_s[:, 768 * k + 512 + 128 * m : 768 * k + 640 + 128 * m],
                            h1[:, 512 * k : 512 * k + 512],
                            k == 0,
                            k == 1,
                        )
                for m in (0, 1):
                    for k in (0, 1):
                        mm(
                            pin[m][:],
                            wihT_s[:, 768 * k + 512 + 128 * m : 768 * k + 640 + 128 * m],
                            cp[:, 512 * k : 512 * k + 512],
                            k == 0,
                            k == 1,
                        )
                rz2 = ap.tile([128, 2048], BF16, name="rz2", tag="rz2")
                for g in range(4):
                    nc.scalar.activation(
                        rz2[:, 512 * g : 512 * g + 512],
                        prz2[g][:],
                        AF.Sigmoid,
                        bias=brz_s[:, g : g + 1],
                    )
                # n2 = tanh(b_ih_n + i_n + r2 * (gh_n + b_hh_n))
                tmp2 = ap.tile([128, 1024], BF16, name="tmp2", tag="tmp2")
                for m, sl in ((0, S0), (1, S1)):
                    nc.vector.scalar_tensor_tensor(
                        tmp2[:, sl], phn[m][:], bhn_s[:, m : m + 1],
                        rz2[:, sl], OP.add, OP.mult,
                    )
                s2 = ap.tile([128, 1024], BF16, name="s2", tag="s2")
                for m, sl in ((0, S0), (1, S1)):
                    nc.vector.tensor_add(s2[:, sl], tmp2[:, sl], pin[m][:])
                n2 = ap.tile([128, 1024], BF16, name="n2", tag="n2")
                for m, sl in ((0, S0), (1, S1)):
                    nc.scalar.activation(
                        n2[:, sl], s2[:, sl], AF.Tanh, bias=binb_s[:, m : m + 1]
                    )
                # h2 = n2 + z2*(h1 - n2)
                d2 = ap.tile([128, 1024], BF16, name="d2", tag="d2")
                nc.vector.tensor_sub(d2[:], h1[:], n2[:])
                m2 = ap.tile([128, 1024], BF16, name="m2", tag="m2")
                nc.gpsimd.tensor_mul(m2[:], rz2[:, 1024:2048], d2[:])
                h2 = ap.tile([128, 1024], BF16, name="h2", tag="h2")
                nc.vector.tensor_add(h2[:], n2[:], m2[:])
                st[t]["h2"] = h2

            def emitC(t):
                h2 = st.pop(t)["h2"]
                r0 = t * R
                pd = ppd.tile([1, 512], F32, name="pd", tag="psd")
                mm(pd[:], decT_s[:, 0:1], h2[:, S0], True, False)
                mm(pd[:], decT_s[:, 1:2], h2[:, S1], False, True)
                ot = io.tile([1, 512], F32, name="ot", tag="ot")
                nc.vector.tensor_scalar_add(ot[:], pd[:], decb_s[0:1, 0:1])
                nc.sync.dma_start(out_ap[0:1, r0 : r0 + R], ot[:])

            emitA(0)
            emitA(1)
            for t in range(nt):
                if t + 2 < nt:
                    emitA(t + 2)
                emitB(t)
                if t >= 1:
                    emitC(t - 1)
            emitC(nt - 1)

    nc.compile()
    return nc


def prep_shared(enc_W, enc_b, fobs_W, fobs_b, W_ih, b_ih, W_hh, b_hh, dec_W, dec_b):
    f = np.float32
    whhT = W_hh.T.astype(f)                      # [256, 768]
    wihT = (W_ih / A).T.astype(f)                # [256, 768], 1/A folded in
    bsum = (b_ih + b_hh).astype(f)
    bf = NP_BF16
    return {
        "encT": np.ascontiguousarray(enc_W.T).astype(bf),                    # [128,256]
        "fobsT": np.ascontiguousarray(
            np.concatenate([fobs_W.T[0:128], fobs_W.T[128:256]], axis=1)
        ).astype(bf),                                                        # [128,512]
        "whhT": np.ascontiguousarray(
            np.concatenate([whhT[0:128], whhT[128:256]], axis=1)
        ).astype(bf),                                                        # [128,1536]
        "wihT": np.ascontiguousarray(
            np.concatenate([wihT[0:128], wihT[128:256]], axis=1)
        ).astype(bf),                                                        # [128,1536]
        "decT": np.ascontiguousarray(
            np.concatenate([dec_W.T[0:128], dec_W.T[128:256]], axis=1)
        ).astype(bf),                                                        # [128,2]
        "encb": np.ascontiguousarray(enc_b.reshape(2, 128).T.astype(f)),
        "fobsb": np.ascontiguousarray(fobs_b.reshape(2, 128).T.astype(f)),
        "brz": np.ascontiguousarray(bsum[0:512].reshape(4, 128).T),
        "bhn": np.ascontiguousarray(b_hh[512:768].reshape(2, 128).T.astype(f)),
        "binb": np.ascontiguousarray(b_ih[512:768].reshape(2, 128).T.astype(f)),
        "decb": dec_b.reshape(1, 1).astype(f),
    }


_NC_CACHE = {}


def _get_nc(n_rows):
    if n_rows not in _NC_CACHE:
        _NC_CACHE[n_rows] = build_nc(n_rows)
    return _NC_CACHE[n_rows]


def run(inputs, trace=False):
    """Shard, run on 8 cores, gather. Returns (out [B,A,1] f32, results)."""
    obs = np.asarray(inputs["obs"], dtype=np.float32)
    shared = prep_shared(
        np.asarray(inputs["enc_W"]), np.asarray(inputs["enc_b"]),
        np.asarray(inputs["fobs_W"]), np.asarray(inputs["fobs_b"]),
        np.asarray(inputs["W_ih"]), np.asarray(inputs["b_ih"]),
        np.asarray(inputs["W_hh"]), np.asarray(inputs["b_hh"]),
        np.asarray(inputs["dec_W"]), np.asarray(inputs["dec_b"]),
    )
    in_maps = []
    for c in range(NCORES):
        xT = np.ascontiguousarray(
            obs[c * B_LOC : (c + 1) * B_LOC].reshape(N_LOC, D).T
        ).astype(NP_BF16)
        in_maps.append({"xT": xT, **shared})

    nc = _get_nc(N_LOC)
    res = run_bass_kernel_spmd(nc, in_maps, core_ids=list(range(NCORES)), trace=trace)
    outs = [res.results[c]["out"].reshape(N_LOC) for c in range(NCORES)]
    full = np.concatenate(outs).reshape(B, A, 1).astype(np.float32)
    return full, res


def kernel(**inputs):
    out, _ = run(inputs, trace=False)
    return out


# revision 17
# speedup vs baseline: 1.2427x; 1.0384x over previous
"""CommNet critic forward kernel for 8 trn2 NeuronCores.

Sharding: pure data parallel over the batch dim (B=2048 -> 256 per core).
Weights (<1MB) replicated. The agent-mean communication is within each
sample's 32-agent group, which never crosses a core boundary, so there are
no collectives.

On-chip layout is feature-major: activations live as [feature -> partition,
row -> free-dim] tiles, so
  * every matmul is out = W_T.T @ acts with the weight stationary,
  * biases are per-partition scalars fused into ACT/DVE instructions
    (or rank-1 ones-matmuls accumulated straight into PSUM),
  * the per-sample mean over 32 agents is a free-dim segmented reduction.
obs is transposed host-side (part of sharding prep) so the device never
transposes anything.

All matmul operands are bf16 (fp32 PSUM accumulate): on trn2 this is the
fast PE path (FWL weight loads + LDW/MM overlap); fp32/fp32r matmuls cost
3-4x more per instruction. Elementwise runs bf16 where operands live in
SBUF (2x DVE modes), with the GRU h-update chains offloaded to GpSimd.
"""

import sys

sys.path.insert(0, "/opt/trn_rl_repo")

import ml_dtypes
import numpy as np

import concourse.bacc as bacc
import concourse.mybir as mybir
import concourse.tile as tile
from concourse.bass_utils import run_bass_kernel_spmd

B, A, D, H = 2048, 32, 128, 256
NCORES = 8
B_LOC = B // NCORES          # 256 samples per core
N_LOC = B_LOC * A            # 8192 rows per core
R = 512                      # rows per tile (one PSUM bank of fp32)

F32 = mybir.dt.float32
BF16 = mybir.dt.bfloat16
NP_BF16 = ml_dtypes.bfloat16

AF = mybir.ActivationFunctionType
OP = mybir.AluOpType

S0 = slice(0, 512)
S1 = slice(512, 1024)


def build_nc(n_rows=N_LOC):
    assert n_rows % R == 0
    nt = n_rows // R
    nc = bacc.Bacc("TRN2", target_bir_lowering=False, debug=False)

    xT = nc.declare_dram_parameter("xT", [D, n_rows], BF16, isOutput=False)
    encT = nc.declare_dram_parameter("encT", [128, 256], BF16, isOutput=False)
    fobsT = nc.declare_dram_parameter("fobsT", [128, 512], BF16, isOutput=False)
    whhT = nc.declare_dram_parameter("whhT", [128, 1536], BF16, isOutput=False)
    wihT = nc.declare_dram_parameter("wihT", [128, 1536], BF16, isOutput=False)
    decT = nc.declare_dram_parameter("decT", [128, 2], BF16, isOutput=False)
    encb = nc.declare_dram_parameter("encb", [128, 2], F32, isOutput=False)
    fobsb = nc.declare_dram_parameter("fobsb", [128, 2], F32, isOutput=False)
    brz = nc.declare_dram_parameter("brz", [128, 4], F32, isOutput=False)
    bhn = nc.declare_dram_parameter("bhn", [128, 2], F32, isOutput=False)
    binb = nc.declare_dram_parameter("binb", [128, 2], F32, isOutput=False)
    decb = nc.declare_dram_parameter("decb", [1, 1], F32, isOutput=False)
    out = nc.declare_dram_parameter("out", [1, n_rows], F32, isOutput=True)

    def mm(o, lhsT, rhs, start, stop):
        nc.tensor.matmul(o, lhsT, rhs, start=start, stop=stop)

    with tile.TileContext(nc, pool_alloc_mode="queue") as tc:
        with (
            tc.tile_pool(name="wpool", bufs=1) as wp,
            tc.tile_pool(name="io", bufs=4) as io,
            tc.tile_pool(name="acts", bufs=4) as ap,
            tc.tile_pool(name="psum", bufs=7, space="PSUM") as pp,
            tc.tile_pool(name="psumd", bufs=1, space="PSUM") as ppd,
        ):
            encT_s = wp.tile([128, 256], BF16, name="encT_s", tag="encT_s")
            fobsT_s = wp.tile([128, 512], BF16, name="fobsT_s", tag="fobsT_s")
            whhT_s = wp.tile([128, 1536], BF16, name="whhT_s", tag="whhT_s")
            wihT_s = wp.tile([128, 1536], BF16, name="wihT_s", tag="wihT_s")
            decT_s = wp.tile([128, 2], BF16, name="decT_s", tag="decT_s")
            encb_s = wp.tile([128, 2], F32, name="encb_s", tag="encb_s")
            fobsb_s = wp.tile([128, 2], F32, name="fobsb_s", tag="fobsb_s")
            brz_s = wp.tile([128, 4], F32, name="brz_s", tag="brz_s")
            bhn_s = wp.tile([128, 2], F32, name="bhn_s", tag="bhn_s")
            binb_s = wp.tile([128, 2], F32, name="binb_s", tag="binb_s")
            decb_s = wp.tile([1, 1], F32, name="decb_s", tag="decb_s")
            for t, d in [
                (encT_s, encT), (fobsT_s, fobsT), (whhT_s, whhT),
                (wihT_s, wihT), (decT_s, decT), (encb_s, encb),
                (fobsb_s, fobsb), (brz_s, brz), (bhn_s, bhn),
                (binb_s, binb), (decb_s, decb),
            ]:
                nc.sync.dma_start(t[:], d.ap())

            xT_ap = xT.ap()
            out_ap = out.ap()

            # Software pipeline, sub-phase interleaved. Per outer step the
            # PE stream is:
            #   pe(t+2) | GRU2-rz(t) | ph(t+2) | GRU2-n(t) | GRU1(t+2) | dec(t-1)
            # so B(t)'s matmul blocks (inputs two phases old) pad the drain
            # latencies inside A(t+2)'s serial enc->fobs->GRU1 chain, the PE
            # never idles long enough for HAM to re-throttle, and (with FIFO
            # slot recycling) every PSUM slot reuse is either cross-iteration
            # or coincident with a data dependency.
            st = {}

            def emitA1(t):
                # enc: e = relu(enc_W @ x + enc_b)
                r0 = t * R
                xt = io.tile([128, R], BF16, name="xt", tag="xt")
                nc.sync.dma_start(xt[:], xT_ap[:, r0 : r0 + R])
                pe = [pp.tile([128, 512], F32, name=f"pe{m}", tag="ps") for m in (0, 1)]
                for m in (0, 1):
                    mm(pe[m][:], encT_s[:, 128 * m : 128 * m + 128], xt[:], True, True)
                e = ap.tile([128, 1024], BF16, name="e", tag="e")
                for m, sl in ((0, S0), (1, S1)):
                    nc.scalar.activation(
                        e[:, sl], pe[m][:], AF.Relu, bias=encb_s[:, m : m + 1]
                    )
                st[t] = {"e": e}

            def emitA2(t):
                # fobs: h0 = fobs_W @ e + fobs_b
                e = st[t].pop("e")
                ph = [pp.tile([128, 512], F32, name=f"ph{m}", tag="ps") for m in (0, 1)]
                for m in (0, 1):
                    for k in (0, 1):
                        mm(
                            ph[m][:],
                            fobsT_s[:, 256 * k + 128 * m : 256 * k + 128 * m + 128],
                            e[:, 512 * k : 512 * k + 512],
                            k == 0,
                            k == 1,
                        )
                h0 = ap.tile([128, 1024], BF16, name="h0", tag="h0")
                for m, sl in ((0, S0), (1, S1)):
                    nc.vector.tensor_scalar_add(h0[:, sl], ph[m][:], fobsb_s[:, m : m + 1])
                st[t]["h0"] = h0

            def emitA3(t):
                # GRU1 (x = 0 so gi = b_ih) + comm mean
                h0 = st[t].pop("h0")
                prz = [pp.tile([128, 512], F32, name=f"prz{g}", tag="ps") for g in range(4)]
                for g in range(4):
                    for k in (0, 1):
                        mm(
                            prz[g][:],
                            whhT_s[:, 768 * k + 128 * g : 768 * k + 128 * g + 128],
                            h0[:, 512 * k : 512 * k + 512],
                            k == 0,
                            k == 1,
                        )
                pn = [pp.tile([128, 512], F32, name=f"pn{m}", tag="ps") for m in (0, 1)]
                for m in (0, 1):
                    for k in (0, 1):
                        mm(
                            pn[m][:],
                            whhT_s[:, 768 * k + 512 + 128 * m : 768 * k + 640 + 128 * m],
                            h0[:, 512 * k : 512 * k + 512],
                            k == 0,
                            k == 1,
                        )
                rz1 = ap.tile([128, 2048], BF16, name="rz1", tag="rz1")
                for g in range(4):
                    nc.scalar.activation(
                        rz1[:, 512 * g : 512 * g + 512],
                        prz[g][:],
                        AF.Sigmoid,
                        bias=brz_s[:, g : g + 1],
                    )
                # n1 = tanh(b_ih_n + r1 * (gh_n + b_hh_n))
                tmp1 = ap.tile([128, 1024], BF16, name="tmp1", tag="tmp1")
                for m, sl in ((0, S0), (1, S1)):
                    nc.vector.scalar_tensor_tensor(
                        tmp1[:, sl], pn[m][:], bhn_s[:, m : m + 1],
                        rz1[:, sl], OP.add, OP.mult,
                    )
                n1 = ap.tile([128, 1024], BF16, name="n1", tag="n1")
                for m, sl in ((0, S0), (1, S1)):
                    nc.scalar.activation(
                        n1[:, sl], tmp1[:, sl], AF.Tanh, bias=binb_s[:, m : m + 1]
                    )
                # h1 = n1 + z1*(h0 - n1)
                d1 = ap.tile([128, 1024], BF16, name="d1", tag="d1")
                nc.vector.tensor_sub(d1[:], h0[:], n1[:])
                m1 = ap.tile([128, 1024], BF16, name="m1", tag="m1")
                nc.gpsimd.tensor_mul(m1[:], rz1[:, 1024:2048], d1[:])
                h1 = ap.tile([128, 1024], BF16, name="h1", tag="h1", bufs=6)
                nc.vector.tensor_add(h1[:], n1[:], m1[:])
                # comm: c' = (sum_group h1) - h1  (1/A folded into W_ih)
                S = ap.tile([128, 32], F32, name="S", tag="S")
                nc.vector.tensor_reduce(
                    S[:].rearrange("p (h g) -> p h g", h=2),
                    h1[:].rearrange("p (h g a) -> p h g a", h=2, a=32),
                    mybir.AxisListType.X,
                    OP.add,
                )
                cp = ap.tile([128, 1024], BF16, name="cp", tag="cp", bufs=6)
                Sb = (
                    S[:]
                    .rearrange("p (h g) -> p h g", h=2)
                    .unsqueeze(-1)
                    .broadcast_to([128, 2, 16, 32])
                )
                nc.gpsimd.tensor_tensor(
                    cp[:].rearrange("p (h g a) -> p h g a", h=2, a=32),
                    Sb,
                    h1[:].rearrange("p (h g a) -> p h g a", h=2, a=32),
                    OP.subtract,
                )
                st[t]["h1"] = h1
                st[t]["cp"] = cp

            def emitBrz(t):
                # GRU2 r,z gates: gi + gh accumulated in one PSUM group
                h1, cp = st[t]["h1"], st[t]["cp"]
                prz2 = [pp.tile([128, 512], F32, name=f"prz2{g}", tag="ps") for g in range(4)]
                for g in range(4):
                    w0 = 128 * g
                    mm(prz2[g][:], wihT_s[:, w0 : w0 + 128], cp[:, S0], True, False)
                    mm(prz2[g][:], wihT_s[:, 768 + w0 : 768 + w0 + 128], cp[:, S1], False, False)
                    mm(prz2[g][:], whhT_s[:, w0 : w0 + 128], h1[:, S0], False, False)
                    mm(prz2[g][:], whhT_s[:, 768 + w0 : 768 + w0 + 128], h1[:, S1], False, True)
                rz2 = ap.tile([128, 2048], BF16, name="rz2", tag="rz2")
                for g in range(4):
                    nc.scalar.activation(
                        rz2[:, 512 * g : 512 * g + 512],
                        prz2[g][:],
                        AF.Sigmoid,
                        bias=brz_s[:, g : g + 1],
                    )
                st[t]["rz2"] = rz2

            def emitBn(t):
                # GRU2 n gate + h2
                h1, cp, rz2 = st[t]["h1"], st[t]["cp"], st[t]["rz2"]
                phn = [pp.tile([128, 512], F32, name=f"phn{m}", tag="ps") for m in (0, 1)]
                pin = [pp.tile([128, 512], F32, name=f"pin{m}", tag="ps") for m in (0, 1)]
                for m in (0, 1):
                    for k in (0, 1):
                        mm(
                            phn[m][:],
                            whhT_s[:, 768 * k + 512 + 128 * m : 768 * k + 640 + 128 * m],
                            h1[:, 512 * k : 512 * k + 512],
                            k == 0,
                            k == 1,
                        )
                for m in (0, 1):
                    for k in (0, 1):
                        mm(
                            pin[m][:],
                            wihT_s[:, 768 * k + 512 + 128 * m : 768 * k + 640 + 128 * m],
                            cp[:, 512 * k : 512 * k + 512],
                            k == 0,
                            k == 1,
                        )
                # n2 = tanh(b_ih_n + i_n + r2 * (gh_n + b_hh_n))
                tmp2 = ap.tile([128, 1024], BF16, name="tmp2", tag="tmp2")
                for m, sl in ((0, S0), (1, S1)):
                    nc.vector.scalar_tensor_tensor(
                        tmp2[:, sl], phn[m][:], bhn_s[:, m : m + 1],
                        rz2[:, sl], OP.add, OP.mult,
                    )
                s2 = ap.tile([128, 1024], BF16, name="s2", tag="s2")
                for m, sl in ((0, S0), (1, S1)):
                    nc.vector.tensor_add(s2[:, sl], tmp2[:, sl], pin[m][:])
                n2 = ap.tile([128, 1024], BF16, name="n2", tag="n2")
                for m, sl in ((0, S0), (1, S1)):
                    nc.scalar.activation(
                        n2[:, sl], s2[:, sl], AF.Tanh, bias=binb_s[:, m : m + 1]
                    )
                # h2 = n2 + z2*(h1 - n2)
                d2 = ap.tile([128, 1024], BF16, name="d2", tag="d2")
                nc.vector.tensor_sub(d2[:], h1[:], n2[:])
                m2 = ap.tile([128, 1024], BF16, name="m2", tag="m2")
                nc.gpsimd.tensor_mul(m2[:], rz2[:, 1024:2048], d2[:])
                h2 = ap.tile([128, 1024], BF16, name="h2", tag="h2")
                nc.vector.tensor_add(h2[:], n2[:], m2[:])
                st[t]["h2"] = h2

            def emitC(t):
                h2 = st.pop(t)["h2"]
                r0 = t * R
                pd = ppd.tile([1, 512], F32, name="pd", tag="psd")
                mm(pd[:], decT_s[:, 0:1], h2[:, S0], True, False)
                mm(pd[:], decT_s[:, 1:2], h2[:, S1], False, True)
                ot = io.tile([1, 512], F32, name="ot", tag="ot")
                nc.vector.tensor_scalar_add(ot[:], pd[:], decb_s[0:1, 0:1])
                nc.sync.dma_start(out_ap[0:1, r0 : r0 + R], ot[:])

            for f in (emitA1, emitA2, emitA3):
                f(0)
                f(1)
            for t in range(nt):
                if t + 2 < nt:
                    emitA1(t + 2)
                emitBrz(t)
                if t + 2 < nt:
                    emitA2(t + 2)
                emitBn(t)
                if t + 2 < nt:
                    emitA3(t + 2)
                if t >= 1:
                    emitC(t - 1)
            emitC(nt - 1)

    nc.compile()
    return nc


def prep_shared(enc_W, enc_b, fobs_W, fobs_b, W_ih, b_ih, W_hh, b_hh, dec_W, dec_b):
    f = np.float32
    whhT = W_hh.T.astype(f)                      # [256, 768]
    wihT = (W_ih / A).T.astype(f)                # [256, 768], 1/A folded in
    bsum = (b_ih + b_hh).astype(f)
    bf = NP_BF16
    return {
        "encT": np.ascontiguousarray(enc_W.T).astype(bf),                    # [128,256]
        "fobsT": np.ascontiguousarray(
            np.concatenate([fobs_W.T[0:128], fobs_W.T[128:256]], axis=1)
        ).astype(bf),                                                        # [128,512]
        "whhT": np.ascontiguousarray(
            np.concatenate([whhT[0:128], whhT[128:256]], axis=1)
        ).astype(bf),                                                        # [128,1536]
        "wihT": np.ascontiguousarray(
            np.concatenate([wihT[0:128], wihT[128:256]], axis=1)
        ).astype(bf),                                                        # [128,1536]
        "decT": np.ascontiguousarray(
            np.concatenate([dec_W.T[0:128], dec_W.T[128:256]], axis=1)
        ).astype(bf),                                                        # [128,2]
        "encb": np.ascontiguousarray(enc_b.reshape(2, 128).T.astype(f)),
        "fobsb": np.ascontiguousarray(fobs_b.reshape(2, 128).T.astype(f)),
        "brz": np.ascontiguousarray(bsum[0:512].reshape(4, 128).T),
        "bhn": np.ascontiguousarray(b_hh[512:768].reshape(2, 128).T.astype(f)),
        "binb": np.ascontiguousarray(b_ih[512:768].reshape(2, 128).T.astype(f)),
        "decb": dec_b.reshape(1, 1).astype(f),
    }


_NC_CACHE = {}


def _get_nc(n_rows):
    if n_rows not in _NC_CACHE:
        _NC_CACHE[n_rows] = build_nc(n_rows)
    return _NC_CACHE[n_rows]


def run(inputs, trace=False):
    """Shard, run on 8 cores, gather. Returns (out [B,A,1] f32, results)."""
    obs = np.asarray(inputs["obs"], dtype=np.float32)
    shared = prep_shared(
        np.asarray(inputs["enc_W"]), np.asarray(inputs["enc_b"]),
        np.asarray(inputs["fobs_W"]), np.asarray(inputs["fobs_b"]),
        np.asarray(inputs["W_ih"]), np.asarray(inputs["b_ih"]),
        np.asarray(inputs["W_hh"]), np.asarray(inputs["b_hh"]),
        np.asarray(inputs["dec_W"]), np.asarray(inputs["dec_b"]),
    )
    in_maps = []
    for c in range(NCORES):
        xT = np.ascontiguousarray(
            obs[c * B_LOC : (c + 1) * B_LOC].reshape(N_LOC, D).T
        ).astype(NP_BF16)
        in_maps.append({"xT": xT, **shared})

    nc = _get_nc(N_LOC)
    res = run_bass_kernel_spmd(nc, in_maps, core_ids=list(range(NCORES)), trace=trace)
    outs = [res.results[c]["out"].reshape(N_LOC) for c in range(NCORES)]
    full = np.concatenate(outs).reshape(B, A, 1).astype(np.float32)
    return full, res


def kernel(**inputs):
    out, _ = run(inputs, trace=False)
    return out
